# revision 24
# baseline (speedup 1.0000x reference)
"""Trainium2 Bass kernel for nn_CausalSelfAttention_59253368815644.

Sharding: 8 cores = 2 (batch) x 4 (head groups of 4 heads). Each core
computes qkv projection + rms-norm + rotary in a transposed [hd, t] layout
(PE-assisted cross-partition reductions), KEY_OFFSET band shift (free-axis
DMA shifts), doc-masked causal attention (doc mask fused into the score
matmul through one-hot augmented contraction rows; causal via affine_select
on boundary tiles; softmax without max-subtraction -- scores are bounded by
attn_scale*HD), gated value embedding, attention output gate, and a partial
output projection over its 256 head-dim columns. Host sums 4 partials per
batch element.

v2: bf16 data path everywhere on the matmul/DVE side (FWL weight loads,
2x DVE modes, half the DMA bytes), single activation-table set
(natural_log_exp_and_others: rsqrt = exp(-0.5*ln), sigmoid = 1/(1+exp(-x))
with DVE reciprocal, attention exp native), Vh ones-column via memset
instead of an 8192-descriptor broadcast DMA, and direct DVE writes into Qh.
"""
import sys

sys.path.insert(0, "/opt/trn_rl_repo")

from contextlib import ExitStack

import numpy as np
import ml_dtypes

import concourse.bass as bass
import concourse.tile as tile
from concourse import bacc, mybir
from concourse._compat import with_exitstack
from concourse.bass_utils import run_bass_kernel_spmd

F32 = mybir.dt.float32
BF16 = mybir.dt.bfloat16
AF = mybir.ActivationFunctionType
NPBF16 = ml_dtypes.bfloat16

B, T, D, H, HD = 2, 2048, 1024, 16, 64
EPS = 1.1920929e-07
VE_GATE_SCALE = 2.0
NHEADS = 4          # heads per core
HGROUPS = 4
NCHUNK = D // 128   # 8 contraction chunks
TTILE = 512
NTT = T // TTILE
BIG = 30.0          # mask exponent after exp-scale
NDOC = 8
AUG = NDOC + 1
QR = 64 + AUG       # 73 partitions for Q^/K^


def build_spans(segs):
    """Greedy partition of [0,T) into q-spans (len 256..512 where possible),
    preferring doc-boundary ends. Returns [(a, b, kts)] with kts = the
    128-aligned k tiles covering [doc_start(a), b)."""
    bounds = [e for (_, e) in segs]
    spans = []
    a = 0
    while a < T:
        cands = [e for e in bounds if a < e <= a + 512]
        end = None
        if cands:
            mx = max(cands)
            if mx - a >= 256 or mx == T:
                end = mx
        if end is None:
            end = min(a + 512, T)
        if end % 2 != 0 and end < T:
            end += 1
        ks = max((s for (s, _) in segs if s <= a), default=0)
        spans.append((a, end, ks))
        a = end
    out = []
    for (a, b, ks) in spans:
        ka0 = (ks // 128) * 128
        kts = []
        ka = ka0
        while ka < b:
            kn = min(128, b - ka)
            kts.append((ka, kn, (ka + kn) > a))
            ka += 128
        out.append((a, b, kts))
    return out


@with_exitstack
def build_kernel(ctx: ExitStack, tc: tile.TileContext, dr, spans, alpha):
    nc = tc.nc

    const = ctx.enter_context(tc.tile_pool(name="const", bufs=1))
    persist = ctx.enter_context(tc.tile_pool(name="persist", bufs=1))

    # small consts first on scalar (PE warm-up deps), bulk spread on queues
    e2b4 = const.tile([128, 128], BF16)
    nc.scalar.dma_start(e2b4[:], dr["e2b4"][:])
    ones2 = const.tile([128, 2], BF16)
    nc.scalar.dma_start(ones2[:], dr["ones2"][:])
    wqk = const.tile([128, NCHUNK, 512], BF16)
    nc.scalar.dma_start(wqk[:],
                        dr["wqk"][:].rearrange("p (c e) -> p c e", e=512))
    cdup = const.tile([128, T], BF16)
    nc.scalar.dma_start(cdup[:], dr["cdup"][:])
    s2shuf = const.tile([128, T], BF16)
    nc.scalar.dma_start(s2shuf[:], dr["s2shuf"][:])
    e4a = const.tile([4, 128], BF16)
    nc.scalar.dma_start(e4a[:], dr["e4"][0, :, :])
    e4b = const.tile([4, 128], BF16)
    nc.scalar.dma_start(e4b[:], dr["e4"][1, :, :])
    ve2 = const.tile([128, T // 128, 256], BF16)
    nc.scalar.dma_start(ve2[:],
                        dr["ve2"][:].rearrange("p (k e) -> p k e", e=256))
    wv = const.tile([128, NCHUNK, 260], BF16)
    nc.gpsimd.dma_start(wv[:],
                        dr["wv"][:].rearrange("p (c e) -> p c e", e=260))
    wga = const.tile([128, NCHUNK, NHEADS], BF16)
    nc.gpsimd.dma_start(wga[:],
                        dr["wga"][:].rearrange("p (c e) -> p c e", e=NHEADS))
    epsb = const.tile([128, 1], F32)
    nc.vector.memset(epsb[:], EPS)

    # full x in SBUF (bf16, 32KB/partition), 4 block DMAs
    xf = const.tile([128, NTT, NCHUNK, TTILE], BF16)
    for tt in range(NTT):
        eng = (nc.sync, nc.gpsimd, nc.sync, nc.gpsimd)[tt]
        eng.dma_start(
            xf[:, tt, :, :],
            dr["xT"][:, tt * NCHUNK * TTILE:(tt + 1) * NCHUNK * TTILE]
            .rearrange("p (c t) -> p c t", t=TTILE))
    wo = const.tile([128, 2, 1024], BF16)
    nc.sync.dma_start(wo[:],
                      dr["wo"][:].rearrange("p (c e) -> p c e", e=1024))

    Qh = persist.tile([QR, NHEADS, T], BF16)
    Kh = persist.tile([QR, NHEADS, T], BF16)
    qaug = dr["qaug"]
    kaug = dr["kaug"]
    nc.gpsimd.dma_start(
        Qh[64:QR, :, :],
        bass.AP(tensor=qaug.tensor, offset=qaug.offset,
                ap=[[T, AUG], [0, NHEADS], [1, T]]))
    nc.gpsimd.dma_start(
        Kh[64:QR, :, :],
        bass.AP(tensor=kaug.tensor, offset=kaug.offset,
                ap=[[T, AUG], [0, NHEADS], [1, T]]))
    Vh = persist.tile([128, T // 128, NHEADS, 65], BF16)
    # softmax-denominator ones column: strided memset, no DMA
    nc.vector.memset(Vh[:].rearrange("p k h o -> p (k h) o")[:, :, 64:65], 1.0)
    agrow = persist.tile([NHEADS, T], F32)
    y01 = persist.tile([128, T], BF16)
    y23 = persist.tile([128, T], BF16)

    # one PSUM pool for all phases: big (qk proj / scores / o-proj) x3,
    # ss x1, misc (ag / rstd-bcast / gate-bcast / v) x2, yps x2  == 8 banks
    ps = ctx.enter_context(tc.tile_pool(name="ps", bufs=1, space="PSUM"))
    sb = ctx.enter_context(tc.tile_pool(name="sbw", bufs=2))

    # PE warm-up: dense dummy matmuls during the initial HBM loads so the
    # HAM clock gate is at 8/8 when real work arrives (depends only on the
    # small consts loaded first).
    wps = ps.tile([128, TTILE], F32, tag="big", bufs=3)
    for i in range(120):
        nc.tensor.matmul(wps[:, 0:128], e2b4[:], e2b4[:],
                         start=True, stop=True)

    # ---------- interleaved main loop ----------
    # Emission order is per-engine execution order; the structure below
    # software-pipelines cross-engine latency: PV matmuls trail score
    # matmuls by one k-tile, span finalization (denominators / gating)
    # is deferred until after the next span's matmuls, and o-proj
    # trails behind that.
    emitted_spans = 0
    emitted_ti = 0
    pending_final = None
    pending_oproj = []

    def emit_span_mms(a, b_, kts):
        N = b_ - a
        ypss = [None] * NHEADS
        for hp in range(2):
            pair = (2 * hp, 2 * hp + 1)
            for h in pair:
                ypss[h] = ps.tile([65, 512], F32, tag=f"y{h % 2}", bufs=1,
                                  name=f"yps{h}")
            hist = []
            nk = len(kts)
            for ki, (ka, kn, causal) in enumerate(kts):
                w0 = max(0, ka - a)
                cur = {}
                for h in pair:
                    sps = ps.tile([128, 512], F32, tag="big", bufs=3)
                    nc.tensor.matmul(sps[0:kn, w0:N],
                                     Kh[:, h, ka:ka + kn],
                                     Qh[:, h, a + w0:b_],
                                     start=True, stop=True)
                    pt = sb.tile([128, 512], BF16, tag="p", bufs=6)
                    nc.scalar.activation(out=pt[0:kn, w0:N],
                                         in_=sps[0:kn, w0:N],
                                         func=AF.Exp, scale=alpha)
                    if causal:
                        bw = min(N, ka + kn - a) - w0
                        if bw > 0:
                            nc.gpsimd.affine_select(
                                out=pt[0:kn, w0:w0 + bw],
                                in_=pt[0:kn, w0:w0 + bw],
                                compare_op=mybir.AluOpType.is_ge,
                                fill=0.0, base=a + w0 - ka,
                                pattern=[[1, bw]], channel_multiplier=-1)
                    cur[h] = (pt, kn, w0, ka)
                hist.append(cur)
                # PV trails the score stream by 2 k-tiles to hide exp latency
                if ki >= 2:
                    ent = hist[ki - 2]
                    for h in pair:
                        pp, pkn, pw0, pka = ent[h]
                        nc.tensor.matmul(ypss[h][:, pw0:N],
                                         Vh[0:pkn, pka // 128, h, :],
                                         pp[0:pkn, pw0:N],
                                         start=(ki == 2), stop=False)
            for ki in range(max(0, nk - 2), nk):
                ent = hist[ki]
                for h in pair:
                    pp, pkn, pw0, pka = ent[h]
                    nc.tensor.matmul(ypss[h][:, pw0:N],
                                     Vh[0:pkn, pka // 128, h, :],
                                     pp[0:pkn, pw0:N],
                                     start=(ki == 0),
                                     stop=(ki == nk - 1))
        return (a, b_, N, ypss)

    def emit_span_final(st):
        a, b_, N, ypss = st
        # softmax denominators -> one [4, N] tile via DVE copies + DMA
        l4 = sb.tile([NHEADS, 512], F32, tag="l4")
        for h in range(NHEADS):
            l1 = sb.tile([1, 512], F32, tag=f"l1_{h}")
            nc.vector.tensor_copy(l1[:, 0:N], ypss[h][64:65, 0:N])
            nc.sync.dma_start(l4[h:h + 1, 0:N], l1[:, 0:N])
        rl4 = sb.tile([NHEADS, 512], F32, tag="rl")
        nc.vector.reciprocal_approx_fast(out=rl4[:, 0:N], in_=l4[:, 0:N])
        sc4 = sb.tile([NHEADS, 512], BF16, tag="sc")
        nc.vector.tensor_mul(sc4[:, 0:N], rl4[:, 0:N], agrow[:, a:b_])
        sbcs = []
        for pr in range(2):
            sbc = ps.tile([128, 512], F32, tag="misc", bufs=2)
            sbcs.append(sbc)
            nc.tensor.matmul(sbc[:, 0:N], e4a[:] if pr == 0 else e4b[:],
                             sc4[:, 0:N], start=True, stop=True)
        yys = []
        for pr in range(2):
            yy = sb.tile([128, 512], BF16, tag="yy")
            yys.append(yy)
            nc.vector.tensor_copy(yy[0:64, 0:N], ypss[2 * pr][0:64, 0:N])
            nc.scalar.activation(out=yy[64:128, 0:N],
                                 in_=ypss[2 * pr + 1][0:64, 0:N],
                                 func=AF.Copy, scale=1.0)
        for pr, ytile in ((0, y01), (1, y23)):
            nc.vector.tensor_mul(ytile[:, a:b_], yys[pr][:, 0:N],
                                 sbcs[pr][:, 0:N])

    def emit_oproj(ti):
        tt0 = ti * 128
        osb = sb.tile([128, 1024], BF16, tag="osb", bufs=2)
        for eh in range(2):
            ops = ps.tile([128, 512], F32, tag="big", bufs=3)
            nc.tensor.matmul(ops[:], y01[:, tt0:tt0 + 128],
                             wo[:, 0, eh * 512:(eh + 1) * 512],
                             start=True, stop=False)
            nc.tensor.matmul(ops[:], y23[:, tt0:tt0 + 128],
                             wo[:, 1, eh * 512:(eh + 1) * 512],
                             start=False, stop=True)
            if eh == 0:
                nc.scalar.activation(out=osb[:, 0:512], in_=ops[:],
                                     func=AF.Copy, scale=1.0)
            else:
                nc.vector.tensor_copy(osb[:, 512:1024], ops[:])
        nc.sync.dma_start(dr["out"][tt0:tt0 + 128, :], osb[:])

    for tt in range(NTT):
        t0 = tt * TTILE
        xsb = xf[:, tt, :, :]

        # ---- qk projection, per-head pipeline: mms -> sq -> ln -> exp ->
        # rstd broadcast -> rotary (reads qk PSUM directly; no evac copy) ----
        ssps = ps.tile([128, TTILE], F32, tag="ss", bufs=1)
        lt8 = sb.tile([128, TTILE], BF16, tag="lt8")
        rstd8 = sb.tile([128, TTILE], BF16, tag="rstd8")
        prev_rbps = None
        for blk in range(NHEADS):
            h = blk
            qk = ps.tile([128, TTILE], F32, tag="big", bufs=3)
            for c in range(NCHUNK):
                nc.tensor.matmul(
                    qk[:], wqk[:, c, blk * 128:(blk + 1) * 128],
                    xsb[:, c, :],
                    start=(c == 0), stop=(c == NCHUNK - 1))
            sq = sb.tile([128, TTILE], BF16, tag="sq", bufs=4)
            nc.scalar.activation(out=sq[:], in_=qk[:], func=AF.Square,
                                 scale=1.0)
            nc.tensor.matmul(ssps[32 * blk:32 * blk + 2, :], ones2[:],
                             sq[:], start=True, stop=True,
                             tile_position=(0, 32 * blk))
            nc.scalar.activation(out=lt8[32 * blk:32 * blk + 2, :],
                                 in_=ssps[32 * blk:32 * blk + 2, :],
                                 func=AF.Ln, scale=1.0 / HD,
                                 bias=epsb[32 * blk:32 * blk + 2, :])
            nc.scalar.activation(out=rstd8[32 * blk:32 * blk + 2, :],
                                 in_=lt8[32 * blk:32 * blk + 2, :],
                                 func=AF.Exp, scale=-0.5)
            rbps = ps.tile([128, TTILE], F32, tag="misc", bufs=2)
            nc.tensor.matmul(rbps[:], e2b4[32 * blk:32 * blk + 2, :],
                             rstd8[32 * blk:32 * blk + 2, :],
                             start=True, stop=True,
                             tile_position=(32 * blk, 0))
            # rotary straight from PSUM: qks = shuffle(qk), A = qk*cos,
            # B = qks * (pre-shuffled sin table)
            qks = sb.tile([128, TTILE], F32, tag="qks")
            nc.vector.stream_shuffle(qks[:], qk[:],
                                     mask=[g ^ 16 for g in range(32)])
            A = sb.tile([128, TTILE], BF16, tag="A")
            nc.vector.tensor_mul(A[:], qk[:], cdup[:, t0:t0 + TTILE])
            Bt = sb.tile([128, TTILE], BF16, tag="B")
            nc.gpsimd.tensor_mul(Bt[:], qks[:], s2shuf[:, t0:t0 + TTILE])
            rotr = sb.tile([128, TTILE], BF16, tag="rotr")
            nc.vector.tensor_add(rotr[:], A[:], Bt[:])
            rot = sb.tile([128, TTILE], BF16, tag="rot")
            nc.vector.tensor_mul(rot[:], rotr[:], rbps[:])
            nc.sync.dma_start(Qh[0:64, h, t0:t0 + TTILE], rot[0:64, :])
            nc.sync.dma_start(Kh[0:32, h, t0:t0 + TTILE], rot[64:96, :])
            w = TTILE if t0 + TTILE < T else TTILE - 1
            nc.sync.dma_start(Kh[32:64, h, t0 + 1:t0 + 1 + w],
                              rot[96:128, 0:w])
            if t0 == 0:
                nc.sync.dma_start(Kh[32:64, h, 0:1], rot[96:128, 0:1])

        # ---- v projection + ve gating ----
        for sub in range(TTILE // 128):
            st = t0 + sub * 128
            vps = ps.tile([128, 512], F32, tag="misc", bufs=2)
            for c in range(NCHUNK):
                nc.tensor.matmul(
                    vps[:, 0:260], xsb[:, c, sub * 128:(sub + 1) * 128],
                    wv[:, c, :],
                    start=(c == 0), stop=(c == NCHUNK - 1))
            gex = sb.tile([128, NHEADS], F32, tag="gex")
            nc.scalar.activation(out=gex[:], in_=vps[:, 256:260],
                                 func=AF.Exp, scale=-1.0)
            gex1 = sb.tile([128, NHEADS], F32, tag="gex1")
            nc.vector.tensor_scalar_add(gex1[:], gex[:], 1.0)
            g = sb.tile([128, NHEADS], F32, tag="g")
            nc.vector.reciprocal_approx_fast(out=g[:], in_=gex1[:])
            gap = g[:]
            gb = bass.AP(tensor=gap.tensor, offset=gap.offset,
                         ap=[list(gap.ap[0]), [1, NHEADS], [0, HD]])
            tmp = sb.tile([128, 256], BF16, tag="vtmp")
            nc.gpsimd.tensor_mul(
                tmp[:].rearrange("p (h d) -> p h d", h=NHEADS),
                ve2[:, st // 128, :].rearrange("p (h d) -> p h d",
                                               h=NHEADS), gb)
            nc.vector.tensor_add(
                Vh[:, st // 128, :, 0:64],
                vps[:, 0:256].rearrange("p (h d) -> p h d", h=NHEADS),
                tmp[:].rearrange("p (h d) -> p h d", h=NHEADS))

        # ---- attn-gate projection (needed only by span finalization) ----
        agps = ps.tile([NHEADS, TTILE], F32, tag="ss", bufs=1)
        for c in range(NCHUNK):
            nc.tensor.matmul(agps[:], wga[:, c, :], xsb[:, c, :],
                             start=(c == 0), stop=(c == NCHUNK - 1))
        eg = sb.tile([NHEADS, TTILE], F32, tag="eg")
        nc.scalar.activation(out=eg[:], in_=agps[:], func=AF.Exp,
                             scale=-1.0)
        eg1 = sb.tile([NHEADS, TTILE], F32, tag="eg1")
        nc.vector.tensor_scalar_add(eg1[:], eg[:], 1.0)
        nc.vector.reciprocal_approx_fast(
            out=agrow[:, t0:t0 + TTILE], in_=eg1[:])

        # ---- attention spans that are now computable (pipelined) ----
        ready = (tt + 1) * TTILE
        while (emitted_spans < len(spans)
               and spans[emitted_spans][1] <= ready):
            a, b_, kts = spans[emitted_spans]
            st = emit_span_mms(a, b_, kts)
            if pending_final is not None:
                emit_span_final(pending_final)
            for ti in pending_oproj:
                emit_oproj(ti)
            pending_oproj = []
            emitted_spans += 1
            # o-proj blocks covered once the *pending* span finalizes
            cover = pending_final[1] if pending_final is not None else 0
            while (emitted_ti + 1) * 128 <= cover:
                pending_oproj.append(emitted_ti)
                emitted_ti += 1
            pending_final = st

    # ---- drain the pipeline ----
    if pending_final is not None:
        emit_span_final(pending_final)
    for ti in pending_oproj:
        emit_oproj(ti)
    while (emitted_ti + 1) * 128 <= T:
        emit_oproj(emitted_ti)
        emitted_ti += 1


_CACHE = {}
TRACE = False       # set by test harness to capture an NTFF profile
LAST_RESULT = None  # BassKernelResults of the most recent run


def _get_program(key, spans, alpha):
    if key in _CACHE:
        return _CACHE[key]
    nc = bacc.Bacc("TRN2", target_bir_lowering=False, debug=False)
    dr = {}

    def di(name, shape, dt=BF16):
        dr[name] = nc.dram_tensor(name, shape, dt, kind="ExternalInput").ap()

    di("xT", [128, NTT * NCHUNK * TTILE])
    di("ve2", [128, (T // 128) * 256])
    di("wqk", [128, NCHUNK * 512])
    di("wv", [128, NCHUNK * 260])
    di("wga", [128, NCHUNK * NHEADS])
    di("wo", [128, 2 * 1024])
    di("cdup", [128, T])
    di("s2shuf", [128, T])
    di("qaug", [AUG, T])
    di("kaug", [AUG, T])
    di("ones2", [128, 2])
    di("e4", [2, 4, 128])
    di("e2b4", [128, 128])
    dr["out"] = nc.dram_tensor("out", [T, D], BF16, kind="ExternalOutput").ap()
    with tile.TileContext(nc) as tc:
        build_kernel(tc, dr, spans, alpha)
    nc.compile()
    _CACHE[key] = nc
    return nc


def kernel(x, ve, sa_lambdas, cos, sin, qkvo_w, attn_gate_w, ve_gate_w,
           attn_scale, docs):
    x = np.asarray(x, dtype=np.float32)
    ve = np.asarray(ve, dtype=np.float32)
    sa_lambdas = np.asarray(sa_lambdas, dtype=np.float32)
    cos = np.asarray(cos, dtype=np.float32)
    sin = np.asarray(sin, dtype=np.float32)
    qkvo_w = np.asarray(qkvo_w, dtype=np.float32)
    attn_gate_w = np.asarray(attn_gate_w, dtype=np.float32)
    ve_gate_w = np.asarray(ve_gate_w, dtype=np.float32)
    docs = np.asarray(docs, dtype=np.int32)
    alpha = float(np.asarray(attn_scale))

    segs = []
    s = 0
    for t in range(1, T + 1):
        if t == T or docs[t] != docs[t - 1]:
            segs.append((s, t))
            s = t
    spans = build_spans(segs)
    nc = _get_program((tuple(segs), alpha), spans, alpha)

    lam0, lam1 = float(sa_lambdas[0]), float(sa_lambdas[1])

    def b16(a):
        return np.ascontiguousarray(a).astype(NPBF16)

    cosT = np.ascontiguousarray(cos.T)
    sinT = np.ascontiguousarray(sin.T)
    cblk = np.concatenate([cosT[0:16], cosT[0:16], cosT[16:32], cosT[16:32]],
                          axis=0)
    sblk = np.concatenate([-sinT[0:16], sinT[0:16], -sinT[16:32],
                           sinT[16:32]], axis=0)
    cdup = np.tile(cblk, (2, 1)).astype(np.float32)
    s2dup = np.tile(sblk, (2, 1)).astype(np.float32)
    s2shuf = s2dup[np.arange(128) ^ 16]
    onehot = (docs[None, :] == np.arange(NDOC)[:, None]).astype(np.float32)
    kaug = np.concatenate([onehot, np.ones((1, T), np.float32)], axis=0)
    qaug = np.concatenate(
        [(BIG / alpha) * onehot, -(BIG / alpha) * np.ones((1, T), np.float32)],
        axis=0).astype(np.float32)
    ones2 = np.zeros((128, 2), np.float32)
    ones2[0:64, 0] = 1.0
    ones2[64:128, 1] = 1.0
    e2b4_host = np.zeros((128, 128), np.float32)
    for _b in range(4):
        e2b4_host[32 * _b, 0:64] = 1.0
        e2b4_host[32 * _b + 1, 64:128] = 1.0
    e4 = np.zeros((2, 4, 128), np.float32)
    e4[0, 0, 0:64] = 1.0
    e4[0, 1, 64:128] = 1.0
    e4[1, 2, 0:64] = 1.0
    e4[1, 3, 64:128] = 1.0

    Wq, Wk, Wv, Wo = (qkvo_w[0:D], qkvo_w[D:2 * D], qkvo_w[2 * D:3 * D],
                      qkvo_w[3 * D:4 * D])

    in_maps = []
    for core in range(8):
        b = core // HGROUPS
        hg = core % HGROUPS
        heads = list(range(hg * NHEADS, (hg + 1) * NHEADS))
        perm = np.r_[0:16, 32:48, 16:32, 48:64]
        blocks = []
        for h in heads:
            blocks.append(lam0 * Wq[h * HD:(h + 1) * HD][perm].T)
            blocks.append(lam0 * Wk[h * HD:(h + 1) * HD][perm].T)
        wqk = np.concatenate(blocks, axis=1).astype(np.float32)
        wqk = np.ascontiguousarray(
            wqk.reshape(NCHUNK, 128, 512).transpose(1, 0, 2).reshape(128, -1))
        wv_cols = [lam0 * Wv[h * HD:(h + 1) * HD].T for h in heads]
        wv_cols.append(ve_gate_w[heads].T)
        wv = np.concatenate(wv_cols, axis=1).astype(np.float32)
        wv = np.ascontiguousarray(
            wv.reshape(NCHUNK, 128, 260).transpose(1, 0, 2).reshape(128, -1))
        wga = attn_gate_w[heads].T.astype(np.float32)
        wga = np.ascontiguousarray(
            wga.reshape(NCHUNK, 128, NHEADS).transpose(1, 0, 2).reshape(128, -1))
        wo = (lam1 * Wo[:, hg * 256:(hg + 1) * 256].T).astype(np.float32)
        wo = np.ascontiguousarray(
            wo.reshape(2, 128, 1024).transpose(1, 0, 2).reshape(128, -1))
        xTn = x[b].T.astype(np.float32)  # [D, T]
        # [p, (tau c t)] layout: per-tau contiguous rows
        xT = np.ascontiguousarray(
            xTn.reshape(NCHUNK, 128, NTT, TTILE).transpose(1, 2, 0, 3)
            .reshape(128, -1))
        # token-major ve blocks: [128, nk*256], token t = k*128 + p
        ve_sl = VE_GATE_SCALE * ve[b, :, hg * 256:(hg + 1) * 256]
        ve2 = np.ascontiguousarray(
            ve_sl.reshape(T // 128, 128, 256).transpose(1, 0, 2)
            .reshape(128, -1))
        in_maps.append({
            "xT": b16(xT), "ve2": b16(ve2), "wqk": b16(wqk), "wv": b16(wv),
            "wga": b16(wga), "wo": b16(wo), "cdup": b16(cdup),
            "s2shuf": b16(s2shuf), "qaug": b16(qaug), "kaug": b16(kaug),
            "ones2": b16(ones2), "e4": b16(e4), "e2b4": b16(e2b4_host),
        })

    global LAST_RESULT
    res = run_bass_kernel_spmd(nc, in_maps, list(range(8)), trace=TRACE)
    LAST_RESULT = res
    out = np.zeros((B, T, D), dtype=np.float32)
    for core in range(8):
        out[core // HGROUPS] += res.results[core]["out"].astype(np.float32)
    return out


# revision 25
# speedup vs baseline: 1.0026x; 1.0026x over previous
"""Trainium2 Bass kernel for nn_CausalSelfAttention_59253368815644.

Sharding: 8 cores = 2 (batch) x 4 (head groups of 4 heads). Each core
computes qkv projection + rms-norm + rotary in a transposed [hd, t] layout
(PE-assisted cross-partition reductions), KEY_OFFSET band shift (free-axis
DMA shifts), doc-masked causal attention (doc mask fused into the score
matmul through one-hot augmented contraction rows; causal via affine_select
on boundary tiles; softmax without max-subtraction -- scores are bounded by
attn_scale*HD), gated value embedding, attention output gate, and a partial
output projection over its 256 head-dim columns. Host sums 4 partials per
batch element.

v2: bf16 data path everywhere on the matmul/DVE side (FWL weight loads,
2x DVE modes, half the DMA bytes), single activation-table set
(natural_log_exp_and_others: rsqrt = exp(-0.5*ln), sigmoid = 1/(1+exp(-x))
with DVE reciprocal, attention exp native), Vh ones-column via memset
instead of an 8192-descriptor broadcast DMA, and direct DVE writes into Qh.
"""
import sys

sys.path.insert(0, "/opt/trn_rl_repo")

from contextlib import ExitStack

import numpy as np
import ml_dtypes

import concourse.bass as bass
import concourse.tile as tile
from concourse import bacc, mybir
from concourse._compat import with_exitstack
from concourse.bass_utils import run_bass_kernel_spmd

F32 = mybir.dt.float32
BF16 = mybir.dt.bfloat16
AF = mybir.ActivationFunctionType
NPBF16 = ml_dtypes.bfloat16

B, T, D, H, HD = 2, 2048, 1024, 16, 64
EPS = 1.1920929e-07
VE_GATE_SCALE = 2.0
NHEADS = 4          # heads per core
HGROUPS = 4
NCHUNK = D // 128   # 8 contraction chunks
TTILE = 512
NTT = T // TTILE
BIG = 30.0          # mask exponent after exp-scale
NDOC = 8
AUG = NDOC + 1
QR = 64 + AUG       # 73 partitions for Q^/K^


def build_spans(segs):
    """Greedy partition of [0,T) into q-spans (len 256..512 where possible),
    preferring doc-boundary ends. Returns [(a, b, kts)] with kts = the
    128-aligned k tiles covering [doc_start(a), b)."""
    bounds = [e for (_, e) in segs]
    spans = []
    a = 0
    while a < T:
        cands = [e for e in bounds if a < e <= a + 512]
        end = None
        if cands:
            mx = max(cands)
            if mx - a >= 256 or mx == T:
                end = mx
        if end is None:
            end = min(a + 512, T)
        if end % 2 != 0 and end < T:
            end += 1
        ks = max((s for (s, _) in segs if s <= a), default=0)
        spans.append((a, end, ks))
        a = end
    out = []
    for (a, b, ks) in spans:
        ka0 = (ks // 128) * 128
        kts = []
        ka = ka0
        while ka < b:
            kn = min(128, b - ka)
            kts.append((ka, kn, (ka + kn) > a))
            ka += 128
        out.append((a, b, kts))
    return out


@with_exitstack
def build_kernel(ctx: ExitStack, tc: tile.TileContext, dr, spans, alpha):
    nc = tc.nc

    const = ctx.enter_context(tc.tile_pool(name="const", bufs=1))
    persist = ctx.enter_context(tc.tile_pool(name="persist", bufs=1))

    # small consts first on scalar (PE warm-up deps), bulk spread on queues
    e2b4 = const.tile([128, 128], BF16)
    nc.scalar.dma_start(e2b4[:], dr["e2b4"][:])
    ones2 = const.tile([128, 2], BF16)
    nc.scalar.dma_start(ones2[:], dr["ones2"][:])
    wqk = const.tile([128, NCHUNK, 512], BF16)
    nc.scalar.dma_start(wqk[:],
                        dr["wqk"][:].rearrange("p (c e) -> p c e", e=512))
    cdup = const.tile([128, T], BF16)
    nc.scalar.dma_start(cdup[:], dr["cdup"][:])
    s2shuf = const.tile([128, T], BF16)
    nc.scalar.dma_start(s2shuf[:], dr["s2shuf"][:])
    e4a = const.tile([4, 128], BF16)
    nc.scalar.dma_start(e4a[:], dr["e4"][0, :, :])
    e4b = const.tile([4, 128], BF16)
    nc.scalar.dma_start(e4b[:], dr["e4"][1, :, :])
    ve2 = const.tile([128, T // 128, 256], BF16)
    nc.scalar.dma_start(ve2[:],
                        dr["ve2"][:].rearrange("p (k e) -> p k e", e=256))
    wv = const.tile([128, NCHUNK, 260], BF16)
    nc.gpsimd.dma_start(wv[:],
                        dr["wv"][:].rearrange("p (c e) -> p c e", e=260))
    wga = const.tile([128, NCHUNK, NHEADS], BF16)
    nc.gpsimd.dma_start(wga[:],
                        dr["wga"][:].rearrange("p (c e) -> p c e", e=NHEADS))
    epsb = const.tile([128, 1], F32)
    nc.vector.memset(epsb[:], EPS)

    # full x in SBUF (bf16, 32KB/partition), 4 block DMAs
    xf = const.tile([128, NTT, NCHUNK, TTILE], BF16)
    for tt in range(NTT):
        eng = (nc.sync, nc.gpsimd, nc.sync, nc.gpsimd)[tt]
        eng.dma_start(
            xf[:, tt, :, :],
            dr["xT"][:, tt * NCHUNK * TTILE:(tt + 1) * NCHUNK * TTILE]
            .rearrange("p (c t) -> p c t", t=TTILE))
    wo = const.tile([128, 2, 1024], BF16)
    nc.sync.dma_start(wo[:],
                      dr["wo"][:].rearrange("p (c e) -> p c e", e=1024))

    Qh = persist.tile([QR, NHEADS, T], BF16)
    Kh = persist.tile([QR, NHEADS, T], BF16)
    qaug = dr["qaug"]
    kaug = dr["kaug"]
    nc.gpsimd.dma_start(
        Qh[64:QR, :, :],
        bass.AP(tensor=qaug.tensor, offset=qaug.offset,
                ap=[[T, AUG], [0, NHEADS], [1, T]]))
    nc.gpsimd.dma_start(
        Kh[64:QR, :, :],
        bass.AP(tensor=kaug.tensor, offset=kaug.offset,
                ap=[[T, AUG], [0, NHEADS], [1, T]]))
    Vh = persist.tile([128, T // 128, NHEADS, 65], BF16)
    # softmax-denominator ones column: strided memset, no DMA
    nc.vector.memset(Vh[:].rearrange("p k h o -> p (k h) o")[:, :, 64:65], 1.0)
    agrow = persist.tile([NHEADS, T], F32)
    y01 = persist.tile([128, T], BF16)
    y23 = persist.tile([128, T], BF16)

    # one PSUM pool for all phases: big (qk proj / scores / o-proj) x3,
    # ss x1, misc (ag / rstd-bcast / gate-bcast / v) x2, yps x2  == 8 banks
    ps = ctx.enter_context(tc.tile_pool(name="ps", bufs=1, space="PSUM"))
    sb = ctx.enter_context(tc.tile_pool(name="sbw", bufs=2))

    # PE warm-up: dense dummy matmuls during the initial HBM loads so the
    # HAM clock gate is at 8/8 when real work arrives (depends only on the
    # small consts loaded first).
    wps = ps.tile([128, TTILE], F32, tag="big", bufs=3)
    for i in range(120):
        nc.tensor.matmul(wps[:, 0:128], e2b4[:], e2b4[:],
                         start=True, stop=True)

    # ---------- interleaved main loop ----------
    # Emission order is per-engine execution order; the structure below
    # software-pipelines cross-engine latency: PV matmuls trail score
    # matmuls by one k-tile, span finalization (denominators / gating)
    # is deferred until after the next span's matmuls, and o-proj
    # trails behind that.
    emitted_spans = 0
    emitted_ti = 0
    pending_final = None
    pending_oproj = []

    def emit_span_mms(a, b_, kts):
        N = b_ - a
        ypss = [None] * NHEADS
        for hp in range(2):
            pair = (2 * hp, 2 * hp + 1)
            for h in pair:
                ypss[h] = ps.tile([65, 512], F32, tag=f"y{h % 2}", bufs=1,
                                  name=f"yps{h}")
            hist = []
            nk = len(kts)
            for ki, (ka, kn, causal) in enumerate(kts):
                w0 = max(0, ka - a)
                cur = {}
                for h in pair:
                    sps = ps.tile([128, 512], F32, tag="big", bufs=3)
                    nc.tensor.matmul(sps[0:kn, w0:N],
                                     Kh[:, h, ka:ka + kn],
                                     Qh[:, h, a + w0:b_],
                                     start=True, stop=True)
                    pt = sb.tile([128, 512], BF16, tag="p", bufs=6)
                    nc.scalar.activation(out=pt[0:kn, w0:N],
                                         in_=sps[0:kn, w0:N],
                                         func=AF.Exp, scale=alpha)
                    if causal:
                        bw = min(N, ka + kn - a) - w0
                        if bw > 0:
                            nc.gpsimd.affine_select(
                                out=pt[0:kn, w0:w0 + bw],
                                in_=pt[0:kn, w0:w0 + bw],
                                compare_op=mybir.AluOpType.is_ge,
                                fill=0.0, base=a + w0 - ka,
                                pattern=[[1, bw]], channel_multiplier=-1)
                    cur[h] = (pt, kn, w0, ka)
                hist.append(cur)
                # PV trails the score stream by 2 k-tiles to hide exp latency
                if ki >= 2:
                    ent = hist[ki - 2]
                    for h in pair:
                        pp, pkn, pw0, pka = ent[h]
                        nc.tensor.matmul(ypss[h][:, pw0:N],
                                         Vh[0:pkn, pka // 128, h, :],
                                         pp[0:pkn, pw0:N],
                                         start=(ki == 2), stop=False)
            for ki in range(max(0, nk - 2), nk):
                ent = hist[ki]
                for h in pair:
                    pp, pkn, pw0, pka = ent[h]
                    nc.tensor.matmul(ypss[h][:, pw0:N],
                                     Vh[0:pkn, pka // 128, h, :],
                                     pp[0:pkn, pw0:N],
                                     start=(ki == 0),
                                     stop=(ki == nk - 1))
        return (a, b_, N, ypss)

    def emit_span_final(st):
        a, b_, N, ypss = st
        # softmax denominators -> one [4, N] tile via DVE copies + DMA
        l4 = sb.tile([NHEADS, 512], F32, tag="l4")
        for h in range(NHEADS):
            l1 = sb.tile([1, 512], F32, tag=f"l1_{h}")
            nc.vector.tensor_copy(l1[:, 0:N], ypss[h][64:65, 0:N])
            nc.sync.dma_start(l4[h:h + 1, 0:N], l1[:, 0:N])
        rl4 = sb.tile([NHEADS, 512], F32, tag="rl")
        nc.vector.reciprocal_approx_fast(out=rl4[:, 0:N], in_=l4[:, 0:N])
        sc4 = sb.tile([NHEADS, 512], BF16, tag="sc")
        nc.vector.tensor_mul(sc4[:, 0:N], rl4[:, 0:N], agrow[:, a:b_])
        sbcs = []
        for pr in range(2):
            sbc = ps.tile([128, 512], F32, tag="misc", bufs=2)
            sbcs.append(sbc)
            nc.tensor.matmul(sbc[:, 0:N], e4a[:] if pr == 0 else e4b[:],
                             sc4[:, 0:N], start=True, stop=True)
        yys = []
        for pr in range(2):
            yy = sb.tile([128, 512], BF16, tag="yy")
            yys.append(yy)
            nc.vector.tensor_copy(yy[0:64, 0:N], ypss[2 * pr][0:64, 0:N])
            nc.scalar.activation(out=yy[64:128, 0:N],
                                 in_=ypss[2 * pr + 1][0:64, 0:N],
                                 func=AF.Copy, scale=1.0)
        for pr, ytile in ((0, y01), (1, y23)):
            nc.vector.tensor_mul(ytile[:, a:b_], yys[pr][:, 0:N],
                                 sbcs[pr][:, 0:N])

    def emit_oproj(ti):
        tt0 = ti * 128
        osb = sb.tile([128, 1024], BF16, tag="osb", bufs=2)
        for eh in range(2):
            ops = ps.tile([128, 512], F32, tag="big", bufs=3)
            nc.tensor.matmul(ops[:], y01[:, tt0:tt0 + 128],
                             wo[:, 0, eh * 512:(eh + 1) * 512],
                             start=True, stop=False)
            nc.tensor.matmul(ops[:], y23[:, tt0:tt0 + 128],
                             wo[:, 1, eh * 512:(eh + 1) * 512],
                             start=False, stop=True)
            if eh == 0:
                nc.scalar.activation(out=osb[:, 0:512], in_=ops[:],
                                     func=AF.Copy, scale=1.0)
            else:
                nc.vector.tensor_copy(osb[:, 512:1024], ops[:])
        nc.sync.dma_start(dr["out"][tt0:tt0 + 128, :], osb[:])

    for tt in range(NTT):
        t0 = tt * TTILE
        xsb = xf[:, tt, :, :]

        # ---- qk projection: per-head pipeline, normalization chain for
        # head b runs while head b+1's matmuls stream (one-head skew) ----
        ssps = ps.tile([128, TTILE], F32, tag="ss", bufs=1)
        lt8 = sb.tile([128, TTILE], BF16, tag="lt8")
        rstd8 = sb.tile([128, TTILE], BF16, tag="rstd8")
        qktiles = [None] * NHEADS
        sqtiles = [None] * NHEADS

        def blk_tail(blk):
            h = blk
            qk = qktiles[blk]
            nc.tensor.matmul(ssps[32 * blk:32 * blk + 2, :], ones2[:],
                             sqtiles[blk][:], start=True, stop=True,
                             tile_position=(0, 32 * blk))
            nc.scalar.activation(out=lt8[32 * blk:32 * blk + 2, :],
                                 in_=ssps[32 * blk:32 * blk + 2, :],
                                 func=AF.Ln, scale=1.0 / HD,
                                 bias=epsb[32 * blk:32 * blk + 2, :])
            nc.scalar.activation(out=rstd8[32 * blk:32 * blk + 2, :],
                                 in_=lt8[32 * blk:32 * blk + 2, :],
                                 func=AF.Exp, scale=-0.5)
            rbps = ps.tile([128, TTILE], F32, tag="misc", bufs=2)
            nc.tensor.matmul(rbps[:], e2b4[32 * blk:32 * blk + 2, :],
                             rstd8[32 * blk:32 * blk + 2, :],
                             start=True, stop=True,
                             tile_position=(32 * blk, 0))
            # rotary straight from PSUM: qks = shuffle(qk), A = qk*cos,
            # B = qks * (pre-shuffled sin table)
            qks = sb.tile([128, TTILE], F32, tag="qks")
            nc.vector.stream_shuffle(qks[:], qk[:],
                                     mask=[g ^ 16 for g in range(32)])
            A = sb.tile([128, TTILE], BF16, tag="A")
            nc.vector.tensor_mul(A[:], qk[:], cdup[:, t0:t0 + TTILE])
            Bt = sb.tile([128, TTILE], BF16, tag="B")
            nc.gpsimd.tensor_mul(Bt[:], qks[:], s2shuf[:, t0:t0 + TTILE])
            rotr = sb.tile([128, TTILE], BF16, tag="rotr")
            nc.vector.tensor_add(rotr[:], A[:], Bt[:])
            rot = sb.tile([128, TTILE], BF16, tag="rot")
            nc.vector.tensor_mul(rot[:], rotr[:], rbps[:])
            nc.sync.dma_start(Qh[0:64, h, t0:t0 + TTILE], rot[0:64, :])
            nc.sync.dma_start(Kh[0:32, h, t0:t0 + TTILE], rot[64:96, :])
            w = TTILE if t0 + TTILE < T else TTILE - 1
            nc.sync.dma_start(Kh[32:64, h, t0 + 1:t0 + 1 + w],
                              rot[96:128, 0:w])
            if t0 == 0:
                nc.sync.dma_start(Kh[32:64, h, 0:1], rot[96:128, 0:1])

        def v_sub(sub):
            st = t0 + sub * 128
            vps = ps.tile([128, 512], F32, tag="misc", bufs=2)
            for c in range(NCHUNK):
                nc.tensor.matmul(
                    vps[:, 0:260], xsb[:, c, sub * 128:(sub + 1) * 128],
                    wv[:, c, :],
                    start=(c == 0), stop=(c == NCHUNK - 1))
            gex = sb.tile([128, NHEADS], F32, tag="gex")
            nc.scalar.activation(out=gex[:], in_=vps[:, 256:260],
                                 func=AF.Exp, scale=-1.0)
            gex1 = sb.tile([128, NHEADS], F32, tag="gex1")
            nc.vector.tensor_scalar_add(gex1[:], gex[:], 1.0)
            g = sb.tile([128, NHEADS], F32, tag="g")
            nc.vector.reciprocal_approx_fast(out=g[:], in_=gex1[:])
            gap = g[:]
            gb = bass.AP(tensor=gap.tensor, offset=gap.offset,
                         ap=[list(gap.ap[0]), [1, NHEADS], [0, HD]])
            tmp = sb.tile([128, 256], BF16, tag="vtmp")
            nc.gpsimd.tensor_mul(
                tmp[:].rearrange("p (h d) -> p h d", h=NHEADS),
                ve2[:, st // 128, :].rearrange("p (h d) -> p h d",
                                               h=NHEADS), gb)
            nc.vector.tensor_add(
                Vh[:, st // 128, :, 0:64],
                vps[:, 0:256].rearrange("p (h d) -> p h d", h=NHEADS),
                tmp[:].rearrange("p (h d) -> p h d", h=NHEADS))

        for blk in range(NHEADS):
            qk = ps.tile([128, TTILE], F32, tag="big", bufs=3)
            qktiles[blk] = qk
            for c in range(NCHUNK):
                nc.tensor.matmul(
                    qk[:], wqk[:, c, blk * 128:(blk + 1) * 128],
                    xsb[:, c, :],
                    start=(c == 0), stop=(c == NCHUNK - 1))
            sq = sb.tile([128, TTILE], BF16, tag="sq", bufs=4)
            sqtiles[blk] = sq
            nc.scalar.activation(out=sq[:], in_=qk[:], func=AF.Square,
                                 scale=1.0)
            if blk >= 1:
                blk_tail(blk - 1)
        v_sub(0)
        blk_tail(NHEADS - 1)
        v_sub(1)
        v_sub(2)
        v_sub(3)

        # ---- attn-gate projection (needed only by span finalization) ----
        agps = ps.tile([NHEADS, TTILE], F32, tag="ss", bufs=1)
        for c in range(NCHUNK):
            nc.tensor.matmul(agps[:], wga[:, c, :], xsb[:, c, :],
                             start=(c == 0), stop=(c == NCHUNK - 1))
        eg = sb.tile([NHEADS, TTILE], F32, tag="eg")
        nc.scalar.activation(out=eg[:], in_=agps[:], func=AF.Exp,
                             scale=-1.0)
        eg1 = sb.tile([NHEADS, TTILE], F32, tag="eg1")
        nc.vector.tensor_scalar_add(eg1[:], eg[:], 1.0)
        nc.vector.reciprocal_approx_fast(
            out=agrow[:, t0:t0 + TTILE], in_=eg1[:])

        # ---- attention spans that are now computable (pipelined) ----
        ready = (tt + 1) * TTILE
        while (emitted_spans < len(spans)
               and spans[emitted_spans][1] <= ready):
            a, b_, kts = spans[emitted_spans]
            st = emit_span_mms(a, b_, kts)
            if pending_final is not None:
                emit_span_final(pending_final)
            for ti in pending_oproj:
                emit_oproj(ti)
            pending_oproj = []
            emitted_spans += 1
            # o-proj blocks covered once the *pending* span finalizes
            cover = pending_final[1] if pending_final is not None else 0
            while (emitted_ti + 1) * 128 <= cover:
                pending_oproj.append(emitted_ti)
                emitted_ti += 1
            pending_final = st

    # ---- drain the pipeline ----
    if pending_final is not None:
        emit_span_final(pending_final)
    for ti in pending_oproj:
        emit_oproj(ti)
    while (emitted_ti + 1) * 128 <= T:
        emit_oproj(emitted_ti)
        emitted_ti += 1


_CACHE = {}
TRACE = False       # set by test harness to capture an NTFF profile
LAST_RESULT = None  # BassKernelResults of the most recent run


def _get_program(key, spans, alpha):
    if key in _CACHE:
        return _CACHE[key]
    nc = bacc.Bacc("TRN2", target_bir_lowering=False, debug=False)
    dr = {}

    def di(name, shape, dt=BF16):
        dr[name] = nc.dram_tensor(name, shape, dt, kind="ExternalInput").ap()

    di("xT", [128, NTT * NCHUNK * TTILE])
    di("ve2", [128, (T // 128) * 256])
    di("wqk", [128, NCHUNK * 512])
    di("wv", [128, NCHUNK * 260])
    di("wga", [128, NCHUNK * NHEADS])
    di("wo", [128, 2 * 1024])
    di("cdup", [128, T])
    di("s2shuf", [128, T])
    di("qaug", [AUG, T])
    di("kaug", [AUG, T])
    di("ones2", [128, 2])
    di("e4", [2, 4, 128])
    di("e2b4", [128, 128])
    dr["out"] = nc.dram_tensor("out", [T, D], BF16, kind="ExternalOutput").ap()
    with tile.TileContext(nc) as tc:
        build_kernel(tc, dr, spans, alpha)
    nc.compile()
    _CACHE[key] = nc
    return nc


def kernel(x, ve, sa_lambdas, cos, sin, qkvo_w, attn_gate_w, ve_gate_w,
           attn_scale, docs):
    x = np.asarray(x, dtype=np.float32)
    ve = np.asarray(ve, dtype=np.float32)
    sa_lambdas = np.asarray(sa_lambdas, dtype=np.float32)
    cos = np.asarray(cos, dtype=np.float32)
    sin = np.asarray(sin, dtype=np.float32)
    qkvo_w = np.asarray(qkvo_w, dtype=np.float32)
    attn_gate_w = np.asarray(attn_gate_w, dtype=np.float32)
    ve_gate_w = np.asarray(ve_gate_w, dtype=np.float32)
    docs = np.asarray(docs, dtype=np.int32)
    alpha = float(np.asarray(attn_scale))

    segs = []
    s = 0
    for t in range(1, T + 1):
        if t == T or docs[t] != docs[t - 1]:
            segs.append((s, t))
            s = t
    spans = build_spans(segs)
    nc = _get_program((tuple(segs), alpha), spans, alpha)

    lam0, lam1 = float(sa_lambdas[0]), float(sa_lambdas[1])

    def b16(a):
        return np.ascontiguousarray(a).astype(NPBF16)

    cosT = np.ascontiguousarray(cos.T)
    sinT = np.ascontiguousarray(sin.T)
    cblk = np.concatenate([cosT[0:16], cosT[0:16], cosT[16:32], cosT[16:32]],
                          axis=0)
    sblk = np.concatenate([-sinT[0:16], sinT[0:16], -sinT[16:32],
                           sinT[16:32]], axis=0)
    cdup = np.tile(cblk, (2, 1)).astype(np.float32)
    s2dup = np.tile(sblk, (2, 1)).astype(np.float32)
    s2shuf = s2dup[np.arange(128) ^ 16]
    onehot = (docs[None, :] == np.arange(NDOC)[:, None]).astype(np.float32)
    kaug = np.concatenate([onehot, np.ones((1, T), np.float32)], axis=0)
    qaug = np.concatenate(
        [(BIG / alpha) * onehot, -(BIG / alpha) * np.ones((1, T), np.float32)],
        axis=0).astype(np.float32)
    ones2 = np.zeros((128, 2), np.float32)
    ones2[0:64, 0] = 1.0
    ones2[64:128, 1] = 1.0
    e2b4_host = np.zeros((128, 128), np.float32)
    for _b in range(4):
        e2b4_host[32 * _b, 0:64] = 1.0
        e2b4_host[32 * _b + 1, 64:128] = 1.0
    e4 = np.zeros((2, 4, 128), np.float32)
    e4[0, 0, 0:64] = 1.0
    e4[0, 1, 64:128] = 1.0
    e4[1, 2, 0:64] = 1.0
    e4[1, 3, 64:128] = 1.0

    Wq, Wk, Wv, Wo = (qkvo_w[0:D], qkvo_w[D:2 * D], qkvo_w[2 * D:3 * D],
                      qkvo_w[3 * D:4 * D])

    in_maps = []
    for core in range(8):
        b = core // HGROUPS
        hg = core % HGROUPS
        heads = list(range(hg * NHEADS, (hg + 1) * NHEADS))
        perm = np.r_[0:16, 32:48, 16:32, 48:64]
        blocks = []
        for h in heads:
            blocks.append(lam0 * Wq[h * HD:(h + 1) * HD][perm].T)
            blocks.append(lam0 * Wk[h * HD:(h + 1) * HD][perm].T)
        wqk = np.concatenate(blocks, axis=1).astype(np.float32)
        wqk = np.ascontiguousarray(
            wqk.reshape(NCHUNK, 128, 512).transpose(1, 0, 2).reshape(128, -1))
        wv_cols = [lam0 * Wv[h * HD:(h + 1) * HD].T for h in heads]
        wv_cols.append(ve_gate_w[heads].T)
        wv = np.concatenate(wv_cols, axis=1).astype(np.float32)
        wv = np.ascontiguousarray(
            wv.reshape(NCHUNK, 128, 260).transpose(1, 0, 2).reshape(128, -1))
        wga = attn_gate_w[heads].T.astype(np.float32)
        wga = np.ascontiguousarray(
            wga.reshape(NCHUNK, 128, NHEADS).transpose(1, 0, 2).reshape(128, -1))
        wo = (lam1 * Wo[:, hg * 256:(hg + 1) * 256].T).astype(np.float32)
        wo = np.ascontiguousarray(
            wo.reshape(2, 128, 1024).transpose(1, 0, 2).reshape(128, -1))
        xTn = x[b].T.astype(np.float32)  # [D, T]
        # [p, (tau c t)] layout: per-tau contiguous rows
        xT = np.ascontiguousarray(
            xTn.reshape(NCHUNK, 128, NTT, TTILE).transpose(1, 2, 0, 3)
            .reshape(128, -1))
        # token-major ve blocks: [128, nk*256], token t = k*128 + p
        ve_sl = VE_GATE_SCALE * ve[b, :, hg * 256:(hg + 1) * 256]
        ve2 = np.ascontiguousarray(
            ve_sl.reshape(T // 128, 128, 256).transpose(1, 0, 2)
            .reshape(128, -1))
        in_maps.append({
            "xT": b16(xT), "ve2": b16(ve2), "wqk": b16(wqk), "wv": b16(wv),
            "wga": b16(wga), "wo": b16(wo), "cdup": b16(cdup),
            "s2shuf": b16(s2shuf), "qaug": b16(qaug), "kaug": b16(kaug),
            "ones2": b16(ones2), "e4": b16(e4), "e2b4": b16(e2b4_host),
        })

    global LAST_RESULT
    res = run_bass_kernel_spmd(nc, in_maps, list(range(8)), trace=TRACE)
    LAST_RESULT = res
    out = np.zeros((B, T, D), dtype=np.float32)
    for core in range(8):
        out[core // HGROUPS] += res.results[core]["out"].astype(np.float32)
    return out


# revision 26
# speedup vs baseline: 1.1168x; 1.1139x over previous
"""Trainium2 Bass kernel for nn_CausalSelfAttention_59253368815644.

Sharding: 8 cores = 2 (batch) x 4 (head groups of 4 heads). Each core
computes qkv projection + rms-norm + rotary in a transposed [hd, t] layout
(PE-assisted cross-partition reductions), KEY_OFFSET band shift (free-axis
DMA shifts), doc-masked causal attention (doc mask fused into the score
matmul through one-hot augmented contraction rows; causal via affine_select
on boundary tiles; softmax without max-subtraction -- scores are bounded by
attn_scale*HD), gated value embedding, attention output gate, and a partial
output projection over its 256 head-dim columns. Host sums 4 partials per
batch element.

v2: bf16 data path everywhere on the matmul/DVE side (FWL weight loads,
2x DVE modes, half the DMA bytes), single activation-table set
(natural_log_exp_and_others: rsqrt = exp(-0.5*ln), sigmoid = 1/(1+exp(-x))
with DVE reciprocal, attention exp native), Vh ones-column via memset
instead of an 8192-descriptor broadcast DMA, and direct DVE writes into Qh.
"""
import sys

sys.path.insert(0, "/opt/trn_rl_repo")

from contextlib import ExitStack

import numpy as np
import ml_dtypes

import concourse.bass as bass
import concourse.tile as tile
from concourse import bacc, mybir
from concourse._compat import with_exitstack
from concourse.bass_utils import run_bass_kernel_spmd

F32 = mybir.dt.float32
BF16 = mybir.dt.bfloat16
AF = mybir.ActivationFunctionType
NPBF16 = ml_dtypes.bfloat16

B, T, D, H, HD = 2, 2048, 1024, 16, 64
EPS = 1.1920929e-07
VE_GATE_SCALE = 2.0
NHEADS = 4          # heads per core
HGROUPS = 4
NCHUNK = D // 128   # 8 contraction chunks
TTILE = 512
NTT = T // TTILE
BIG = 30.0          # mask exponent after exp-scale
NDOC = 8
AUG = NDOC + 1
QR = 64 + AUG       # 73 partitions for Q^/K^


def build_spans(segs):
    """Greedy partition of [0,T) into q-spans (len 256..512 where possible),
    preferring doc-boundary ends. Returns [(a, b, kts)] with kts = the
    128-aligned k tiles covering [doc_start(a), b)."""
    bounds = [e for (_, e) in segs]
    spans = []
    a = 0
    while a < T:
        cands = [e for e in bounds if a < e <= a + 512]
        end = None
        if cands:
            mx = max(cands)
            if mx - a >= 256 or mx == T:
                end = mx
        if end is None:
            end = min(a + 512, T)
        if end % 2 != 0 and end < T:
            end += 1
        ks = max((s for (s, _) in segs if s <= a), default=0)
        spans.append((a, end, ks))
        a = end
    out = []
    for (a, b, ks) in spans:
        ka0 = (ks // 128) * 128
        kts = []
        ka = ka0
        while ka < b:
            kn = min(128, b - ka)
            kts.append((ka, kn, (ka + kn) > a))
            ka += 128
        out.append((a, b, kts))
    return out


@with_exitstack
def build_kernel(ctx: ExitStack, tc: tile.TileContext, dr, spans, alpha):
    nc = tc.nc

    const = ctx.enter_context(tc.tile_pool(name="const", bufs=1))
    persist = ctx.enter_context(tc.tile_pool(name="persist", bufs=1))

    # small consts first on scalar (PE warm-up deps), bulk spread on queues
    e2b4 = const.tile([128, 128], BF16)
    nc.scalar.dma_start(e2b4[:], dr["e2b4"][:])
    ones2 = const.tile([128, 2], BF16)
    nc.scalar.dma_start(ones2[:], dr["ones2"][:])
    wqk = const.tile([128, NCHUNK, 512], BF16)
    nc.scalar.dma_start(wqk[:],
                        dr["wqk"][:].rearrange("p (c e) -> p c e", e=512))
    cdup = const.tile([128, T], BF16)
    nc.scalar.dma_start(cdup[:], dr["cdup"][:])
    s2shuf = const.tile([128, T], BF16)
    nc.scalar.dma_start(s2shuf[:], dr["s2shuf"][:])
    e4a = const.tile([4, 128], BF16)
    nc.scalar.dma_start(e4a[:], dr["e4"][0, :, :])
    e4b = const.tile([4, 128], BF16)
    nc.scalar.dma_start(e4b[:], dr["e4"][1, :, :])
    ve2 = const.tile([128, T // 128, 256], BF16)
    nc.scalar.dma_start(ve2[:],
                        dr["ve2"][:].rearrange("p (k e) -> p k e", e=256))
    wv = const.tile([128, NCHUNK, 260], BF16)
    nc.gpsimd.dma_start(wv[:],
                        dr["wv"][:].rearrange("p (c e) -> p c e", e=260))
    wga = const.tile([128, NCHUNK, NHEADS], BF16)
    nc.gpsimd.dma_start(wga[:],
                        dr["wga"][:].rearrange("p (c e) -> p c e", e=NHEADS))
    epsb = const.tile([128, 1], F32)
    nc.vector.memset(epsb[:], EPS)

    # full x in SBUF (bf16, 32KB/partition), 4 block DMAs
    xf = const.tile([128, NTT, NCHUNK, TTILE], BF16)
    for tt in range(NTT):
        eng = (nc.sync, nc.gpsimd, nc.sync, nc.gpsimd)[tt]
        eng.dma_start(
            xf[:, tt, :, :],
            dr["xT"][:, tt * NCHUNK * TTILE:(tt + 1) * NCHUNK * TTILE]
            .rearrange("p (c t) -> p c t", t=TTILE))
    wo = const.tile([128, 2, 1024], BF16)
    nc.sync.dma_start(wo[:],
                      dr["wo"][:].rearrange("p (c e) -> p c e", e=1024))

    Qh = persist.tile([QR, NHEADS, T], BF16)
    Kh = persist.tile([QR, NHEADS, T], BF16)
    qaug = dr["qaug"]
    kaug = dr["kaug"]
    nc.gpsimd.dma_start(
        Qh[64:QR, :, :],
        bass.AP(tensor=qaug.tensor, offset=qaug.offset,
                ap=[[T, AUG], [0, NHEADS], [1, T]]))
    nc.gpsimd.dma_start(
        Kh[64:QR, :, :],
        bass.AP(tensor=kaug.tensor, offset=kaug.offset,
                ap=[[T, AUG], [0, NHEADS], [1, T]]))
    Vh = persist.tile([128, T // 128, NHEADS, 65], BF16)
    # softmax-denominator ones column: strided memset, no DMA
    nc.vector.memset(Vh[:].rearrange("p k h o -> p (k h) o")[:, :, 64:65], 1.0)
    agrow = persist.tile([NHEADS, T], F32)
    y01 = persist.tile([128, T], BF16)
    y23 = persist.tile([128, T], BF16)

    # one PSUM pool for all phases: big (qk proj / scores / o-proj) x3,
    # ss x1, misc (ag / rstd-bcast / gate-bcast / v) x2, yps x2  == 8 banks
    ps = ctx.enter_context(tc.tile_pool(name="ps", bufs=1, space="PSUM"))
    sb = ctx.enter_context(tc.tile_pool(name="sbw", bufs=2))

    # PE warm-up: dense dummy matmuls during the initial HBM loads so the
    # HAM clock gate is at 8/8 when real work arrives (depends only on the
    # small consts loaded first).
    wps = ps.tile([128, TTILE], F32, tag="big", bufs=3)
    for i in range(120):
        nc.tensor.matmul(wps[:, 0:128], e2b4[:], e2b4[:],
                         start=True, stop=True)

    # ---------- interleaved main loop ----------
    # Emission order is per-engine execution order; the structure below
    # software-pipelines cross-engine latency: PV matmuls trail score
    # matmuls by one k-tile, span finalization (denominators / gating)
    # is deferred until after the next span's matmuls, and o-proj
    # trails behind that.
    emitted_spans = 0
    emitted_ti = 0
    pending_final = None
    pending_oproj = []

    def emit_span_mms(a, b_, kts):
        N = b_ - a
        ypss = [None] * NHEADS
        for hp in range(2):
            pair = (2 * hp, 2 * hp + 1)
            for h in pair:
                ypss[h] = ps.tile([65, 512], F32, tag=f"y{h % 2}", bufs=1,
                                  name=f"yps{h}")
            hist = []
            nk = len(kts)
            for ki, (ka, kn, causal) in enumerate(kts):
                w0 = max(0, ka - a)
                cur = {}
                for h in pair:
                    sps = ps.tile([128, 512], F32, tag="big", bufs=3)
                    nc.tensor.matmul(sps[0:kn, w0:N],
                                     Kh[:, h, ka:ka + kn],
                                     Qh[:, h, a + w0:b_],
                                     start=True, stop=True)
                    pt = sb.tile([128, 512], BF16, tag="p", bufs=6)
                    nc.scalar.activation(out=pt[0:kn, w0:N],
                                         in_=sps[0:kn, w0:N],
                                         func=AF.Exp, scale=alpha)
                    if causal:
                        bw = min(N, ka + kn - a) - w0
                        if bw > 0:
                            nc.gpsimd.affine_select(
                                out=pt[0:kn, w0:w0 + bw],
                                in_=pt[0:kn, w0:w0 + bw],
                                compare_op=mybir.AluOpType.is_ge,
                                fill=0.0, base=a + w0 - ka,
                                pattern=[[1, bw]], channel_multiplier=-1)
                    cur[h] = (pt, kn, w0, ka)
                hist.append(cur)
                # PV trails the score stream by 2 k-tiles to hide exp latency
                if ki >= 2:
                    ent = hist[ki - 2]
                    for h in pair:
                        pp, pkn, pw0, pka = ent[h]
                        nc.tensor.matmul(ypss[h][:, pw0:N],
                                         Vh[0:pkn, pka // 128, h, :],
                                         pp[0:pkn, pw0:N],
                                         start=(ki == 2), stop=False)
            for ki in range(max(0, nk - 2), nk):
                ent = hist[ki]
                for h in pair:
                    pp, pkn, pw0, pka = ent[h]
                    nc.tensor.matmul(ypss[h][:, pw0:N],
                                     Vh[0:pkn, pka // 128, h, :],
                                     pp[0:pkn, pw0:N],
                                     start=(ki == 0),
                                     stop=(ki == nk - 1))
        return (a, b_, N, ypss)

    def emit_span_final(st):
        a, b_, N, ypss = st
        # softmax denominators -> one [4, N] tile via DVE copies + DMA
        l4 = sb.tile([NHEADS, 512], F32, tag="l4")
        for h in range(NHEADS):
            l1 = sb.tile([1, 512], F32, tag=f"l1_{h}")
            nc.vector.tensor_copy(l1[:, 0:N], ypss[h][64:65, 0:N])
            nc.sync.dma_start(l4[h:h + 1, 0:N], l1[:, 0:N])
        rl4 = sb.tile([NHEADS, 512], F32, tag="rl")
        nc.vector.reciprocal_approx_fast(out=rl4[:, 0:N], in_=l4[:, 0:N])
        sc4 = sb.tile([NHEADS, 512], BF16, tag="sc")
        nc.vector.tensor_mul(sc4[:, 0:N], rl4[:, 0:N], agrow[:, a:b_])
        sbcs = []
        for pr in range(2):
            sbc = ps.tile([128, 512], F32, tag="misc", bufs=2)
            sbcs.append(sbc)
            nc.tensor.matmul(sbc[:, 0:N], e4a[:] if pr == 0 else e4b[:],
                             sc4[:, 0:N], start=True, stop=True)
        yys = []
        for pr in range(2):
            yy = sb.tile([128, 512], BF16, tag="yy")
            yys.append(yy)
            nc.vector.tensor_copy(yy[0:64, 0:N], ypss[2 * pr][0:64, 0:N])
            nc.scalar.activation(out=yy[64:128, 0:N],
                                 in_=ypss[2 * pr + 1][0:64, 0:N],
                                 func=AF.Copy, scale=1.0)
        for pr, ytile in ((0, y01), (1, y23)):
            nc.vector.tensor_mul(ytile[:, a:b_], yys[pr][:, 0:N],
                                 sbcs[pr][:, 0:N])

    def emit_oproj(ti):
        tt0 = ti * 128
        osb = sb.tile([128, 1024], BF16, tag="osb", bufs=2)
        for eh in range(2):
            ops = ps.tile([128, 512], F32, tag="big", bufs=3)
            nc.tensor.matmul(ops[:], y01[:, tt0:tt0 + 128],
                             wo[:, 0, eh * 512:(eh + 1) * 512],
                             start=True, stop=False)
            nc.tensor.matmul(ops[:], y23[:, tt0:tt0 + 128],
                             wo[:, 1, eh * 512:(eh + 1) * 512],
                             start=False, stop=True)
            if eh == 0:
                nc.scalar.activation(out=osb[:, 0:512], in_=ops[:],
                                     func=AF.Copy, scale=1.0)
            else:
                nc.vector.tensor_copy(osb[:, 512:1024], ops[:])
        nc.sync.dma_start(dr["out"][tt0:tt0 + 128, :], osb[:])

    for tt in range(NTT):
        t0 = tt * TTILE
        xsb = xf[:, tt, :, :]

        # ---- qk projection matmuls (dense PE stream), evac on ACT ----
        qkes = []
        sqs = []
        for blk in range(NHEADS):
            qk = ps.tile([128, TTILE], F32, tag="big", bufs=3)
            for c in range(NCHUNK):
                nc.tensor.matmul(
                    qk[:], wqk[:, c, blk * 128:(blk + 1) * 128],
                    xsb[:, c, :],
                    start=(c == 0), stop=(c == NCHUNK - 1))
            qke = sb.tile([128, TTILE], BF16, tag="qke", bufs=4)
            qkes.append(qke)
            nc.scalar.activation(out=qke[:], in_=qk[:], func=AF.Copy,
                                 scale=1.0)
            sq = sb.tile([128, TTILE], BF16, tag="sq", bufs=4)
            sqs.append(sq)
            nc.scalar.activation(out=sq[:], in_=qk[:], func=AF.Square,
                                 scale=1.0)
        # ---- sum of squares into one bank (sq's ready by now) ----
        ssps = ps.tile([128, TTILE], F32, tag="ss", bufs=1)
        for blk in range(NHEADS):
            nc.tensor.matmul(ssps[32 * blk:32 * blk + 2, :], ones2[:],
                             sqs[blk][:], start=True, stop=True,
                             tile_position=(0, 32 * blk))
        # rstd = exp(-0.5 ln(ss/HD + eps)), one sweep for 4 heads
        lt8 = sb.tile([128, TTILE], BF16, tag="lt8")
        nc.scalar.activation(out=lt8[:], in_=ssps[:], func=AF.Ln,
                             scale=1.0 / HD, bias=epsb[:])
        rstd8 = sb.tile([128, TTILE], BF16, tag="rstd8")
        nc.scalar.activation(out=rstd8[:], in_=lt8[:], func=AF.Exp,
                             scale=-0.5)

        # ---- v projection + ve gating (PE filler during rstd chain) ----
        for sub in range(TTILE // 128):
            st = t0 + sub * 128
            vps = ps.tile([128, 512], F32, tag="misc", bufs=2)
            for c in range(NCHUNK):
                nc.tensor.matmul(
                    vps[:, 0:260], xsb[:, c, sub * 128:(sub + 1) * 128],
                    wv[:, c, :],
                    start=(c == 0), stop=(c == NCHUNK - 1))
            gex = sb.tile([128, NHEADS], F32, tag="gex")
            nc.scalar.activation(out=gex[:], in_=vps[:, 256:260],
                                 func=AF.Exp, scale=-1.0)
            gex1 = sb.tile([128, NHEADS], F32, tag="gex1")
            nc.vector.tensor_scalar_add(gex1[:], gex[:], 1.0)
            g = sb.tile([128, NHEADS], F32, tag="g")
            nc.vector.reciprocal_approx_fast(out=g[:], in_=gex1[:])
            gap = g[:]
            gb = bass.AP(tensor=gap.tensor, offset=gap.offset,
                         ap=[list(gap.ap[0]), [1, NHEADS], [0, HD]])
            tmp = sb.tile([128, 256], BF16, tag="vtmp")
            nc.gpsimd.tensor_mul(
                tmp[:].rearrange("p (h d) -> p h d", h=NHEADS),
                ve2[:, st // 128, :].rearrange("p (h d) -> p h d",
                                               h=NHEADS), gb)
            nc.vector.tensor_add(
                Vh[:, st // 128, :, 0:64],
                vps[:, 0:256].rearrange("p (h d) -> p h d", h=NHEADS),
                tmp[:].rearrange("p (h d) -> p h d", h=NHEADS))

        # ---- rotary + normalize, per head (shuffle-first variant) ----
        for blk in range(NHEADS):
            h = blk
            qke = qkes[blk]
            rbps = ps.tile([128, TTILE], F32, tag="misc", bufs=2)
            nc.tensor.matmul(rbps[:], e2b4[32 * blk:32 * blk + 2, :],
                             rstd8[32 * blk:32 * blk + 2, :],
                             start=True, stop=True,
                             tile_position=(32 * blk, 0))
            qksb = sb.tile([128, TTILE], BF16, tag="qksb")
            nc.vector.stream_shuffle(qksb[:], qke[:],
                                     mask=[g ^ 16 for g in range(32)])
            A = sb.tile([128, TTILE], BF16, tag="A")
            nc.vector.tensor_mul(A[:], qke[:], cdup[:, t0:t0 + TTILE])
            Bt = sb.tile([128, TTILE], BF16, tag="B")
            nc.gpsimd.tensor_mul(Bt[:], qksb[:], s2shuf[:, t0:t0 + TTILE])
            rotr = sb.tile([128, TTILE], BF16, tag="rotr")
            nc.vector.tensor_add(rotr[:], A[:], Bt[:])
            rot = sb.tile([128, TTILE], BF16, tag="rot")
            nc.vector.tensor_mul(rot[:], rotr[:], rbps[:])
            nc.sync.dma_start(Qh[0:64, h, t0:t0 + TTILE], rot[0:64, :])
            nc.sync.dma_start(Kh[0:32, h, t0:t0 + TTILE], rot[64:96, :])
            w = TTILE if t0 + TTILE < T else TTILE - 1
            nc.sync.dma_start(Kh[32:64, h, t0 + 1:t0 + 1 + w],
                              rot[96:128, 0:w])
            if t0 == 0:
                nc.sync.dma_start(Kh[32:64, h, 0:1], rot[96:128, 0:1])

        # ---- attn-gate projection (needed only by span finalization) ----
        agps = ps.tile([NHEADS, TTILE], F32, tag="ss", bufs=1)
        for c in range(NCHUNK):
            nc.tensor.matmul(agps[:], wga[:, c, :], xsb[:, c, :],
                             start=(c == 0), stop=(c == NCHUNK - 1))
        eg = sb.tile([NHEADS, TTILE], F32, tag="eg")
        nc.scalar.activation(out=eg[:], in_=agps[:], func=AF.Exp,
                             scale=-1.0)
        eg1 = sb.tile([NHEADS, TTILE], F32, tag="eg1")
        nc.vector.tensor_scalar_add(eg1[:], eg[:], 1.0)
        nc.vector.reciprocal_approx_fast(
            out=agrow[:, t0:t0 + TTILE], in_=eg1[:])

        # ---- attention spans that are now computable (pipelined) ----
        ready = (tt + 1) * TTILE
        while (emitted_spans < len(spans)
               and spans[emitted_spans][1] <= ready):
            a, b_, kts = spans[emitted_spans]
            st = emit_span_mms(a, b_, kts)
            if pending_final is not None:
                emit_span_final(pending_final)
            for ti in pending_oproj:
                emit_oproj(ti)
            pending_oproj = []
            emitted_spans += 1
            # o-proj blocks covered once the *pending* span finalizes
            cover = pending_final[1] if pending_final is not None else 0
            while (emitted_ti + 1) * 128 <= cover:
                pending_oproj.append(emitted_ti)
                emitted_ti += 1
            pending_final = st

    # ---- drain the pipeline ----
    if pending_final is not None:
        emit_span_final(pending_final)
    for ti in pending_oproj:
        emit_oproj(ti)
    while (emitted_ti + 1) * 128 <= T:
        emit_oproj(emitted_ti)
        emitted_ti += 1


_CACHE = {}
TRACE = False       # set by test harness to capture an NTFF profile
LAST_RESULT = None  # BassKernelResults of the most recent run


def _get_program(key, spans, alpha):
    if key in _CACHE:
        return _CACHE[key]
    nc = bacc.Bacc("TRN2", target_bir_lowering=False, debug=False)
    dr = {}

    def di(name, shape, dt=BF16):
        dr[name] = nc.dram_tensor(name, shape, dt, kind="ExternalInput").ap()

    di("xT", [128, NTT * NCHUNK * TTILE])
    di("ve2", [128, (T // 128) * 256])
    di("wqk", [128, NCHUNK * 512])
    di("wv", [128, NCHUNK * 260])
    di("wga", [128, NCHUNK * NHEADS])
    di("wo", [128, 2 * 1024])
    di("cdup", [128, T])
    di("s2shuf", [128, T])
    di("qaug", [AUG, T])
    di("kaug", [AUG, T])
    di("ones2", [128, 2])
    di("e4", [2, 4, 128])
    di("e2b4", [128, 128])
    dr["out"] = nc.dram_tensor("out", [T, D], BF16, kind="ExternalOutput").ap()
    with tile.TileContext(nc) as tc:
        build_kernel(tc, dr, spans, alpha)
    nc.compile()
    _CACHE[key] = nc
    return nc


def kernel(x, ve, sa_lambdas, cos, sin, qkvo_w, attn_gate_w, ve_gate_w,
           attn_scale, docs):
    x = np.asarray(x, dtype=np.float32)
    ve = np.asarray(ve, dtype=np.float32)
    sa_lambdas = np.asarray(sa_lambdas, dtype=np.float32)
    cos = np.asarray(cos, dtype=np.float32)
    sin = np.asarray(sin, dtype=np.float32)
    qkvo_w = np.asarray(qkvo_w, dtype=np.float32)
    attn_gate_w = np.asarray(attn_gate_w, dtype=np.float32)
    ve_gate_w = np.asarray(ve_gate_w, dtype=np.float32)
    docs = np.asarray(docs, dtype=np.int32)
    alpha = float(np.asarray(attn_scale))

    segs = []
    s = 0
    for t in range(1, T + 1):
        if t == T or docs[t] != docs[t - 1]:
            segs.append((s, t))
            s = t
    spans = build_spans(segs)
    nc = _get_program((tuple(segs), alpha), spans, alpha)

    lam0, lam1 = float(sa_lambdas[0]), float(sa_lambdas[1])

    def b16(a):
        return np.ascontiguousarray(a).astype(NPBF16)

    cosT = np.ascontiguousarray(cos.T)
    sinT = np.ascontiguousarray(sin.T)
    cblk = np.concatenate([cosT[0:16], cosT[0:16], cosT[16:32], cosT[16:32]],
                          axis=0)
    sblk = np.concatenate([-sinT[0:16], sinT[0:16], -sinT[16:32],
                           sinT[16:32]], axis=0)
    cdup = np.tile(cblk, (2, 1)).astype(np.float32)
    s2dup = np.tile(sblk, (2, 1)).astype(np.float32)
    s2shuf = s2dup[np.arange(128) ^ 16]
    onehot = (docs[None, :] == np.arange(NDOC)[:, None]).astype(np.float32)
    kaug = np.concatenate([onehot, np.ones((1, T), np.float32)], axis=0)
    qaug = np.concatenate(
        [(BIG / alpha) * onehot, -(BIG / alpha) * np.ones((1, T), np.float32)],
        axis=0).astype(np.float32)
    ones2 = np.zeros((128, 2), np.float32)
    ones2[0:64, 0] = 1.0
    ones2[64:128, 1] = 1.0
    e2b4_host = np.zeros((128, 128), np.float32)
    for _b in range(4):
        e2b4_host[32 * _b, 0:64] = 1.0
        e2b4_host[32 * _b + 1, 64:128] = 1.0
    e4 = np.zeros((2, 4, 128), np.float32)
    e4[0, 0, 0:64] = 1.0
    e4[0, 1, 64:128] = 1.0
    e4[1, 2, 0:64] = 1.0
    e4[1, 3, 64:128] = 1.0

    Wq, Wk, Wv, Wo = (qkvo_w[0:D], qkvo_w[D:2 * D], qkvo_w[2 * D:3 * D],
                      qkvo_w[3 * D:4 * D])

    in_maps = []
    for core in range(8):
        b = core // HGROUPS
        hg = core % HGROUPS
        heads = list(range(hg * NHEADS, (hg + 1) * NHEADS))
        perm = np.r_[0:16, 32:48, 16:32, 48:64]
        blocks = []
        for h in heads:
            blocks.append(lam0 * Wq[h * HD:(h + 1) * HD][perm].T)
            blocks.append(lam0 * Wk[h * HD:(h + 1) * HD][perm].T)
        wqk = np.concatenate(blocks, axis=1).astype(np.float32)
        wqk = np.ascontiguousarray(
            wqk.reshape(NCHUNK, 128, 512).transpose(1, 0, 2).reshape(128, -1))
        wv_cols = [lam0 * Wv[h * HD:(h + 1) * HD].T for h in heads]
        wv_cols.append(ve_gate_w[heads].T)
        wv = np.concatenate(wv_cols, axis=1).astype(np.float32)
        wv = np.ascontiguousarray(
            wv.reshape(NCHUNK, 128, 260).transpose(1, 0, 2).reshape(128, -1))
        wga = attn_gate_w[heads].T.astype(np.float32)
        wga = np.ascontiguousarray(
            wga.reshape(NCHUNK, 128, NHEADS).transpose(1, 0, 2).reshape(128, -1))
        wo = (lam1 * Wo[:, hg * 256:(hg + 1) * 256].T).astype(np.float32)
        wo = np.ascontiguousarray(
            wo.reshape(2, 128, 1024).transpose(1, 0, 2).reshape(128, -1))
        xTn = x[b].T.astype(np.float32)  # [D, T]
        # [p, (tau c t)] layout: per-tau contiguous rows
        xT = np.ascontiguousarray(
            xTn.reshape(NCHUNK, 128, NTT, TTILE).transpose(1, 2, 0, 3)
            .reshape(128, -1))
        # token-major ve blocks: [128, nk*256], token t = k*128 + p
        ve_sl = VE_GATE_SCALE * ve[b, :, hg * 256:(hg + 1) * 256]
        ve2 = np.ascontiguousarray(
            ve_sl.reshape(T // 128, 128, 256).transpose(1, 0, 2)
            .reshape(128, -1))
        in_maps.append({
            "xT": b16(xT), "ve2": b16(ve2), "wqk": b16(wqk), "wv": b16(wv),
            "wga": b16(wga), "wo": b16(wo), "cdup": b16(cdup),
            "s2shuf": b16(s2shuf), "qaug": b16(qaug), "kaug": b16(kaug),
            "ones2": b16(ones2), "e4": b16(e4), "e2b4": b16(e2b4_host),
        })

    global LAST_RESULT
    res = run_bass_kernel_spmd(nc, in_maps, list(range(8)), trace=TRACE)
    LAST_RESULT = res
    out = np.zeros((B, T, D), dtype=np.float32)
    for core in range(8):
        out[core // HGROUPS] += res.results[core]["out"].astype(np.float32)
    return out
